# revision 1
# baseline (speedup 1.0000x reference)
"""Trainium2 Bass kernel for AtomFeaturizer (embedding_lookup, 8 cores).

Strategy: the whole featurizer is expressed as one K=102 contraction per
atom-tile against a fused table:
  - 75 rows: one-hot of the six categorical indices (tables concatenated,
    bias b folded into the E_atom rows)
  - 20 rows: one-hot of the four bond-count slots over counts 0..4
    (E_bond replicated per slot; count==0 rows zeroed -> implements the mask)
  - 7 rows: raw bond counts (x W[3:7]/4) and scalar3 (x W[0:3]) -> the linear
    layer
One-hot features are built on-device: a small "broadcast" matmul replicates
each atom's index value across its section's partitions (S matrix), then a
single DVE tensor_scalar(is_equal) against a per-partition iota constant
produces the one-hot block. The main matmul runs in bf16 with a hi/lo split
of the table (two accumulating matmuls, fp32 PSUM) for ~fp32 accuracy at
1 cycle/row PE speed (fp32 matmul streams at 1/4 rate on TRN2). All per-atom
data is packed host-side into one feature-major bf16 array [17, N] so every
DMA is wide and contiguous.

Data parallel over atoms: 125k per NeuronCore, padded to 126976 = 124*1024 so
each 1024-atom group is 8 main matmuls of M=128 with contiguous weight
slices; atoms interleave as a = 8u + s within a group so the output DMA
writes 4KB contiguous runs per psum partition. DMA issue is spread across the
SP (HWDGE, output) and GPSIMD (SWDGE, input + sbuf copy) sequencers.
"""
import numpy as np
import ml_dtypes
from contextlib import ExitStack

from concourse import bacc, mybir
import concourse.bass as bass
import concourse.tile as tile
from concourse.bass_utils import run_bass_kernel_spmd

BF16 = ml_dtypes.bfloat16
NCORES = 8
N_TOTAL = 1_000_000
D = 128

SEC_BASES = [0, 46, 52, 63, 66, 71]
SEC_SIZES = [46, 6, 11, 3, 5, 4]
K_OH = 95     # 75 categorical one-hot rows + 20 bond one-hot rows
K_MAIN = 102  # + 4 bond-count rows + 3 scalar3 rows

GROUP = 1000  # atoms per PSUM group (one 2-bank psum pair)
TILE = 500    # atoms per broadcast matmul (one psum bank)
SUB = 125     # atoms per main matmul (output psum partitions)
BLOCK = 5000  # atoms per DMA block

# aligned (v2) geometry: per-core atom count padded to a multiple of 1024 so
# groups are 1024 atoms = 8 main matmuls of M=128 with contiguous weight
# slices, and output-DMA runs are 4KB (8 consecutive atoms per psum partition)
GROUP2 = 1024
TILE2 = 512
SUB2 = 128
BLOCK2 = 4096
N_PAD = 126976  # 124 * 1024, for n_core = 125000

_NC_CACHE = {}


def build_consts(E_atom, E_deg, E_chg, E_hyb, E_h, E_chi, E_bond, W, b):
    T = np.zeros((K_MAIN, D), np.float32)
    T[0:46] = E_atom + b[None, :]
    T[46:52] = E_deg
    T[52:63] = E_chg
    T[63:66] = E_hyb
    T[66:71] = E_h
    T[71:75] = E_chi
    for j in range(4):
        for c in range(5):
            T[75 + 5 * j + c] = E_bond[c] if c > 0 else 0.0
    T[95:99] = W[3:7] * 0.25
    T[99:102] = W[0:3]
    T_hi = T.astype(BF16)
    T_lo = (T - T_hi.astype(np.float32)).astype(BF16)
    T2 = np.ascontiguousarray(np.concatenate([T_hi, T_lo], axis=1))

    S = np.zeros((10, K_OH), np.float32)
    for t, (base, size) in enumerate(zip(SEC_BASES, SEC_SIZES)):
        S[4 + t, base:base + size] = 1.0
    for j in range(4):
        S[j, 75 + 5 * j: 75 + 5 * j + 5] = 1.0

    C = np.concatenate([np.arange(s) for s in SEC_SIZES]
                       + [np.arange(5)] * 4).astype(np.float32)
    return T2, S.astype(BF16), np.ascontiguousarray(C[:, None])


def build_packed(atom_idx, degree_idx, charge_idx, hybrid_idx, numh_idx,
                 chiral_idx, bond_counts, scalar3):
    # rows 0..3 bond counts, 4..9 categorical indices (broadcast-mm operand);
    # rows 10..13 bond counts again, 14..16 scalar3 (the 7 linear feature
    # rows, contiguous so one sbuf->sbuf copy fills main_tile[95:102]).
    n = atom_idx.shape[0]
    packed = np.empty((17, n), np.float32)
    packed[0:4] = bond_counts.T
    for i, idx in enumerate([atom_idx, degree_idx, charge_idx, hybrid_idx,
                             numh_idx, chiral_idx]):
        packed[4 + i] = idx
    packed[10:14] = packed[0:4]
    packed[14:17] = scalar3.T
    return packed.astype(BF16)


def build_nc(n_core, block=BLOCK, bufs_bc=3, bufs_main=2, bufs_outs=2,
             bufs_psb=2, bufs_pso=2, passes=1, dbg_out_contig=False):
    key = (n_core, block, bufs_bc, bufs_main, bufs_outs, bufs_psb, bufs_pso,
           passes, dbg_out_contig)
    if key in _NC_CACHE:
        return _NC_CACHE[key]
    assert n_core % block == 0 and block % GROUP == 0
    nblocks = n_core // block
    ngroups = block // GROUP
    bf = mybir.dt.bfloat16
    f32 = mybir.dt.float32

    nc = bacc.Bacc("TRN2", target_bir_lowering=False, debug=False)
    packed_d = nc.dram_tensor("packed", [17, n_core], bf, kind="ExternalInput")
    s_d = nc.dram_tensor("s_mat", [10, K_OH], bf, kind="ExternalInput")
    thi_d = nc.dram_tensor("t_hi", [K_MAIN, D], bf, kind="ExternalInput")
    tlo_d = nc.dram_tensor("t_lo", [K_MAIN, D], bf, kind="ExternalInput")
    cvec_d = nc.dram_tensor("cvec", [K_OH, 1], f32, kind="ExternalInput")
    out_d = nc.dram_tensor("out", [n_core, D], f32, kind="ExternalOutput")

    with tile.TileContext(nc) as tc, ExitStack() as ctx:
        consts = ctx.enter_context(tc.tile_pool(name="consts", bufs=1))
        bc_pool = ctx.enter_context(tc.tile_pool(name="bcast", bufs=bufs_bc))
        main_pool = ctx.enter_context(tc.tile_pool(name="main", bufs=bufs_main))
        outs_pool = ctx.enter_context(tc.tile_pool(name="outs", bufs=bufs_outs))
        psb_pool = ctx.enter_context(
            tc.tile_pool(name="psb", bufs=bufs_psb, space=bass.MemorySpace.PSUM))
        pso_pool = ctx.enter_context(
            tc.tile_pool(name="pso", bufs=bufs_pso, space=bass.MemorySpace.PSUM))

        s_t = consts.tile([10, K_OH], bf)
        nc.sync.dma_start(s_t[:], s_d.ap())
        thi_t = consts.tile([K_MAIN, D], bf)
        nc.sync.dma_start(thi_t[:], thi_d.ap())
        tlo_t = consts.tile([K_MAIN, D], bf)
        nc.sync.dma_start(tlo_t[:], tlo_d.ap())
        cvec_t = consts.tile([K_OH, 1], f32)
        nc.sync.dma_start(cvec_t[:], cvec_d.ap())

        pap = packed_d.ap()
        oap = out_d.ap()
        for blk in range(nblocks * passes):
            blk = blk % nblocks
            bin_t = bc_pool.tile([17, block], bf)
            nc.gpsimd.dma_start(bin_t[:], pap[:, blk * block:(blk + 1) * block])
            main_t = main_pool.tile([K_MAIN, block], bf)
            # linear feature rows come straight from the packed data
            nc.gpsimd.dma_start(main_t[95:102, :], bin_t[10:17, :])
            outs_t = outs_pool.tile([SUB, ngroups * 1024], f32)
            mview = main_t[0:K_MAIN, :].rearrange(
                "k (g t u s) -> k g t u s", g=ngroups, t=2, u=SUB, s=4)
            for g in range(ngroups):
                psb = psb_pool.tile([K_OH, 1024], f32)
                for t in range(2):
                    lo = g * GROUP + t * TILE
                    nc.tensor.matmul(psb[:, t * 512:t * 512 + TILE],
                                     s_t[:, :], bin_t[0:10, lo:lo + TILE],
                                     start=True, stop=True)
                pv = psb[0:K_OH, :].rearrange(
                    "k (t x) -> k t x", t=2, x=512)[:, :, 0:TILE]
                ov = main_t[0:K_OH, g * GROUP:(g + 1) * GROUP].rearrange(
                    "k (t x) -> k t x", t=2, x=TILE)
                nc.vector.tensor_scalar(ov, pv, cvec_t[:, 0:1], None,
                                        mybir.AluOpType.is_equal)
                pso = pso_pool.tile([SUB, 1024], f32)
                for t in range(2):
                    for s in range(4):
                        lhsT = mview[0:K_MAIN, g, t, :, s]
                        col = t * 512 + s * D
                        nc.tensor.matmul(pso[:, col:col + D], lhsT,
                                         thi_t[:, :], start=True, stop=False)
                        nc.tensor.matmul(pso[:, col:col + D], lhsT,
                                         tlo_t[:, :], start=False, stop=True)
                nc.scalar.copy(outs_t[:, g * 1024:(g + 1) * 1024], pso[:, :])
            if dbg_out_contig:
                # timing probe only: contiguous (wrong-layout) output store
                dst = oap[blk * block:(blk + 1) * block, :].rearrange(
                    "(u x) d -> u (x d)", u=SUB, x=block // SUB)
                nc.sync.dma_start(dst, outs_t[0:SUB, :])
            else:
                dst = oap[blk * block:(blk + 1) * block, :].rearrange(
                    "(g t u s) d -> u g t (s d)", g=ngroups, t=2, u=SUB, s=4)
                src = outs_t[0:SUB, :].rearrange(
                    "p (g t x) -> p g t x", g=ngroups, t=2, x=512)
                nc.sync.dma_start(dst, src)
    nc.compile()
    _NC_CACHE[key] = nc
    return nc


def build_nc2(n_pad, block=BLOCK2, bufs_bc=4, bufs_main=3, bufs_outs=3,
              bufs_psb=4, bufs_pso=2, passes=1, out_split=-4, dbg_skip="",
              fuse_hilo=1):
    key = ("v2", n_pad, block, bufs_bc, bufs_main, bufs_outs, bufs_psb,
           bufs_pso, passes, out_split, dbg_skip, fuse_hilo)
    if key in _NC_CACHE:
        return _NC_CACHE[key]
    skip = set(dbg_skip.split(",")) if dbg_skip else set()
    assert n_pad % block == 0 and block % GROUP2 == 0
    nblocks = n_pad // block
    ngroups = block // GROUP2
    bf = mybir.dt.bfloat16
    f32 = mybir.dt.float32

    nc = bacc.Bacc("TRN2", target_bir_lowering=False, debug=False)
    packed_d = nc.dram_tensor("packed", [17, n_pad], bf, kind="ExternalInput")
    s_d = nc.dram_tensor("s_mat", [10, K_OH], bf, kind="ExternalInput")
    t2_d = nc.dram_tensor("t2", [K_MAIN, 2 * D], bf, kind="ExternalInput")
    cvec_d = nc.dram_tensor("cvec", [K_OH, 1], f32, kind="ExternalInput")
    out_d = nc.dram_tensor("out", [n_pad, D], f32, kind="ExternalOutput")

    with tile.TileContext(nc) as tc, ExitStack() as ctx:
        consts = ctx.enter_context(tc.tile_pool(name="consts", bufs=1))
        bc_pool = ctx.enter_context(tc.tile_pool(name="bcast", bufs=bufs_bc))
        main_pool = ctx.enter_context(tc.tile_pool(name="main", bufs=bufs_main))
        outs_pool = ctx.enter_context(tc.tile_pool(name="outs", bufs=bufs_outs))
        psb_pool = ctx.enter_context(
            tc.tile_pool(name="psb", bufs=bufs_psb, space=bass.MemorySpace.PSUM))
        pso_pool = ctx.enter_context(
            tc.tile_pool(name="pso", bufs=bufs_pso, space=bass.MemorySpace.PSUM))

        s_t = consts.tile([10, K_OH], bf)
        nc.sync.dma_start(s_t[:], s_d.ap())
        t2_t = consts.tile([K_MAIN, 2 * D], bf)
        nc.sync.dma_start(t2_t[:], t2_d.ap())
        cvec_t = consts.tile([K_OH, 1], f32)
        nc.sync.dma_start(cvec_t[:], cvec_d.ap())
        t2v = t2_t[0:K_MAIN, :].rearrange("k (h d) -> k h d", h=2, d=D)

        pap = packed_d.ap()
        oap = out_d.ap()
        for blk in range(nblocks * passes):
            blk = blk % nblocks
            bin_t = bc_pool.tile([10, block], bf)
            nc.gpsimd.dma_start(bin_t[:],
                                pap[0:10, blk * block:(blk + 1) * block])
            main_t = main_pool.tile([K_MAIN, block], bf)
            # linear feature rows straight from DRAM: host stores rows 10..16
            # pre-permuted to the (s, u) column order
            nc.gpsimd.dma_start(main_t[95:102, :],
                                pap[10:17, blk * block:(blk + 1) * block])
            outs_t = outs_pool.tile([SUB2, ngroups * GROUP2], f32)
            for g in range(ngroups):
                for t in range(2):
                    # one 512-atom psum bank per broadcast matmul; is_equal
                    # with the (s, u) permutation: feat col s*128+u, u=t*64+x/8
                    psb = psb_pool.tile([K_OH, TILE2], f32)
                    if "bcast" not in skip:
                        lo = g * GROUP2 + t * TILE2
                        nc.tensor.matmul(psb[:, :], s_t[:, :],
                                         bin_t[0:10, lo:lo + TILE2],
                                         start=True, stop=True)
                    if "iseq" not in skip:
                        pv = psb[0:K_OH, :].rearrange(
                            "k (u s) -> k s u", u=SUB2 // 2, s=8)
                        ov = main_t[0:K_OH, g * GROUP2:(g + 1) * GROUP2].rearrange(
                            "k (s t u) -> k t s u", s=8, t=2, u=SUB2 // 2)[:, t]
                        nc.vector.tensor_scalar(ov, pv, cvec_t[:, 0:1], None,
                                                mybir.AluOpType.is_equal)
                pso = pso_pool.tile([SUB2, GROUP2], f32)
                if "mm" not in skip:
                    for s in range(8):
                        lhsT = main_t[0:K_MAIN, g * GROUP2 + s * SUB2:
                                      g * GROUP2 + (s + 1) * SUB2]
                        col = s * D
                        if fuse_hilo:
                            # single matmul streams T_hi then T_lo through one
                            # stationary load; the zero-stride out AP hits the
                            # same PSUM words twice and has_written accumulates
                            out_ap = (pso[:, col:col + D].unsqueeze(1)
                                      .broadcast_to((SUB2, 2, D)))
                            nc.tensor.matmul(out_ap, lhsT, t2v,
                                             start=True, stop=True)
                        else:
                            nc.tensor.matmul(pso[:, col:col + D], lhsT,
                                             t2_t[:, 0:D],
                                             start=True, stop=False)
                            nc.tensor.matmul(pso[:, col:col + D], lhsT,
                                             t2_t[:, D:2 * D],
                                             start=False, stop=True)
                elif "act" not in skip:
                    nc.tensor.matmul(pso[:, 0:D], main_t[0:K_MAIN, 0:SUB2],
                                     t2_t[:, 0:D], start=True, stop=True)
                if "act" not in skip:
                    nc.scalar.copy(outs_t[:, g * GROUP2:(g + 1) * GROUP2],
                                   pso[:, :])
            if "out" in skip:
                dst = oap[blk * block:(blk + 1) * block, :].rearrange(
                    "(u x) d -> u (x d)", u=SUB2, x=block // SUB2)
                nc.sync.dma_start(dst, outs_t[0:SUB2, :])
            else:
                dst = oap[blk * block:(blk + 1) * block, :].rearrange(
                    "(g u s) d -> u g (s d)", g=ngroups, u=SUB2, s=8)
                src = outs_t[0:SUB2, :].rearrange(
                    "p (g x) -> p g x", g=ngroups, x=GROUP2)
                if out_split > 0:
                    engines = [nc.sync, nc.gpsimd, nc.scalar, nc.gpsimd]
                else:  # negative: |out_split| ways, all on the SP/HWDGE queue
                    engines = [nc.sync] * 4
                nsplit = abs(out_split)
                gper = ngroups // nsplit
                for i in range(nsplit):
                    gs = slice(i * gper, (i + 1) * gper)
                    engines[i].dma_start(dst[:, gs, :], src[:, gs, :])
    nc.compile()
    _NC_CACHE[key] = nc
    return nc


def _permute_linear_rows(rows, n_pad):
    g = n_pad // GROUP2
    return np.ascontiguousarray(
        rows.reshape(rows.shape[0], g, SUB2, 8).transpose(0, 1, 3, 2)
        .reshape(rows.shape[0], n_pad))


def _prepare(inputs, aligned=True):
    inputs = {k: np.asarray(v) for k, v in inputs.items()}
    T2, S, C = build_consts(
        inputs['E_atom'].astype(np.float32), inputs['E_deg'].astype(np.float32),
        inputs['E_chg'].astype(np.float32), inputs['E_hyb'].astype(np.float32),
        inputs['E_h'].astype(np.float32), inputs['E_chi'].astype(np.float32),
        inputs['E_bond'].astype(np.float32), inputs['W'].astype(np.float32),
        inputs['b'].astype(np.float32))
    packed = build_packed(
        inputs['atom_idx'], inputs['degree_idx'], inputs['charge_idx'],
        inputs['hybrid_idx'], inputs['numh_idx'], inputs['chiral_idx'],
        inputs['bond_counts'], inputs['scalar3'])
    n = packed.shape[1]
    n_core = n // NCORES
    if aligned:
        n_pad = -(-n_core // BLOCK2) * BLOCK2
    else:
        n_pad = n_core
    in_maps = []
    for c in range(NCORES):
        p = packed[:, c * n_core:(c + 1) * n_core]
        if n_pad != n_core:
            p = np.concatenate(
                [p, np.zeros((17, n_pad - n_core), BF16)], axis=1)
        p = np.ascontiguousarray(p)
        if aligned:
            p[10:17] = _permute_linear_rows(p[10:17], n_pad)
        in_maps.append({
            "packed": p, "s_mat": S, "t2": T2, "cvec": C,
        })
    return n_core, n_pad, in_maps


def _run(inputs, trace=False, aligned=True, **kw):
    n_core, n_pad, in_maps = _prepare(inputs, aligned=aligned)
    nc = build_nc2(n_pad) if aligned else build_nc(n_pad)
    res = run_bass_kernel_spmd(nc, in_maps, list(range(NCORES)), trace=trace, **kw)
    out = np.concatenate(
        [res.results[c]["out"][:n_core] for c in range(NCORES)], axis=0)
    return out, res


def kernel(**inputs) -> np.ndarray:
    out, _ = _run(inputs, trace=False)
    return out


# ---------------------------------------------------------------------------
# Timing harness (not used by kernel()): repeated on-device execution with
# pre-staged inputs and donated zero output buffers, mirroring
# bass2jax.run_bass_via_pjrt's shard_map build.
# ---------------------------------------------------------------------------

def _build_exec(nc, n_cores):
    import jax
    from jax.experimental.shard_map import shard_map
    from jax.sharding import Mesh, PartitionSpec
    from concourse import bass2jax

    bass2jax.install_neuronx_cc_hook()
    partition_name = (nc.partition_id_tensor.name
                      if nc.partition_id_tensor else None)
    in_names, out_names, out_avals = [], [], []
    for alloc in nc.m.functions[0].allocations:
        if not isinstance(alloc, mybir.MemoryLocationSet):
            continue
        name = alloc.memorylocations[0].name
        if alloc.kind == "ExternalInput":
            if name != partition_name:
                in_names.append(name)
        elif alloc.kind == "ExternalOutput":
            out_names.append(name)
            out_avals.append(jax.core.ShapedArray(
                tuple(alloc.tensor_shape), mybir.dt.np(alloc.dtype)))
    n_params = len(in_names)
    all_in = list(in_names + out_names)
    if partition_name is not None:
        all_in.append(partition_name)
    all_in = tuple(all_in)

    def _body(*args):
        operands = list(args)
        if partition_name is not None:
            operands.append(bass2jax.partition_id_tensor())
        outs = bass2jax._bass_exec_p.bind(
            *operands, out_avals=tuple(out_avals), in_names=all_in,
            out_names=tuple(out_names),
            lowering_input_output_aliases=(),
            sim_require_finite=True, sim_require_nnan=True, nc=nc)
        return tuple(outs)

    devices = jax.devices()[:n_cores]
    mesh = Mesh(np.asarray(devices), ("core",))
    nin = n_params + len(out_names)
    donate = tuple(range(n_params, nin))
    sharded = jax.jit(
        shard_map(_body, mesh=mesh, in_specs=(PartitionSpec("core"),) * nin,
                  out_specs=(PartitionSpec("core"),) * len(out_names),
                  check_rep=False),
        donate_argnums=donate, keep_unused=True)
    return sharded, mesh, in_names, out_names, out_avals


def time_nc(nc, in_maps, iters=16):
    import time as _time
    import jax
    from jax.sharding import NamedSharding, PartitionSpec

    sharded, mesh, in_names, out_names, out_avals = _build_exec(nc, NCORES)
    sh = NamedSharding(mesh, PartitionSpec("core"))
    gin = []
    for name in in_names:
        cat = np.concatenate([np.asarray(m[name]) for m in in_maps], axis=0)
        gin.append(jax.device_put(cat, sh))
    zero_sets = []
    for _ in range(iters + 1):
        zero_sets.append([
            jax.device_put(np.zeros((NCORES * av.shape[0], *av.shape[1:]),
                                    av.dtype), sh)
            for av in out_avals])
    r = sharded(*gin, *zero_sets[0])
    jax.block_until_ready(r)
    del r
    t0 = _time.perf_counter()
    rs = [sharded(*gin, *zero_sets[1 + i]) for i in range(iters)]
    jax.block_until_ready(rs)
    dt = _time.perf_counter() - t0
    return dt / iters * 1e9


def time_pair(nc_a, nc_b, in_maps, reps=10):
    """Interleave executions of two kernels; return per-call medians.

    Robust to the multi-ms, drifting axon-relay dispatch overhead: the two
    kernels see the same overhead distribution, so median(b) - median(a)
    estimates the device-time difference."""
    import time as _time
    import jax
    from jax.sharding import NamedSharding, PartitionSpec

    execs = []
    for nc in (nc_a, nc_b):
        sharded, mesh, in_names, out_names, out_avals = _build_exec(nc, NCORES)
        sh = NamedSharding(mesh, PartitionSpec("core"))
        gin = []
        for name in in_names:
            cat = np.concatenate([np.asarray(m[name]) for m in in_maps], axis=0)
            gin.append(jax.device_put(cat, sh))
        zeros = [
            jax.device_put(np.zeros((NCORES * av.shape[0], *av.shape[1:]),
                                    av.dtype), sh)
            for av in out_avals]
        execs.append((sharded, gin, zeros, out_avals, sh))

    def one_call(i):
        sharded, gin, zeros, out_avals, sh = execs[i]
        import jax as _jax
        t0 = _time.perf_counter()
        r = sharded(*gin, *zeros)
        _jax.block_until_ready(r)
        dt = _time.perf_counter() - t0
        # donation consumed the zero buffers; recycle outputs as next zeros
        execs[i] = (sharded, gin, list(r), out_avals, sh)
        return dt

    one_call(0), one_call(1)  # warmup/compile
    ta, tb = [], []
    for _ in range(reps):
        ta.append(one_call(0))
        tb.append(one_call(1))
    ta.sort(), tb.sort()
    med_a = ta[len(ta) // 2] * 1e9
    med_b = tb[len(tb) // 2] * 1e9
    return med_a, med_b


def time_kernel(inputs, iters=16, aligned=True, **kw):
    n_core, n_pad, in_maps = _prepare(inputs, aligned=aligned)
    nc = build_nc2(n_pad, **kw) if aligned else build_nc(n_pad, **kw)
    return time_nc(nc, in_maps, iters)

